# revision 32
# baseline (speedup 1.0000x reference)
"""Fused single-head attention kernel for 8 TRN2 NeuronCores.

Problem: B=4, S=2048, D=1024 attention:
    Q = x @ Wq.T + bq; K = x @ Wk.T + bk; V = x @ Wv.T + bv
    out = softmax(Q K^T / sqrt(D)) @ V

Sharding (no cross-core traffic): core c handles batch b = c//2 and
query half h = c%2 (1024 queries).

Algebraic refactoring: the K/V projections are folded away:
  logits[k,q] = x_k . (M2^T x_q + z)   (M2 = Wq^T Wk / sqrt(D),
                                        z = Wk^T bq / sqrt(D);
  per-query terms drop out of the softmax). out = ((P x) Wv^T)/rowsum(P)
  + bv, so V is never materialized.

The S x QH logits matmul (pass1, the largest single score-path item)
runs in fp8-e4m3 with MatmulPerfMode.DoubleRow (K=256 per matmul,
2 fp8 weights per PE cell), roughly halving its TensorE time. Y itself
is computed in bf16 (fp8 M2/x there pushes rel-err past the 2e-2 gate:
measured 2.1e-2 full-fp8 vs 1.4e-2 with bf16 Y; bf16 baseline 2.3e-3).
The output path (pass2: x^T P, pass3: (.)Wv^T) stays bf16.

fp8 scale management (e4m3 min normal 2^-6): Y has rms ~0.01, so the
Y evacuation scales by 128 (-> rms ~1.3 in fp8; bias z is host-scaled
by 128 and added in the same activation, before quantization). pass1
PSUM then holds 128*logits and the exp evac uses scale 1/128.

DoubleRow data layout: 3D tiles [128, 2, N]; element (p, j, n) is
contraction index (2*dp+j)*128+p for d-tile-pair dp. Both operands use
the same (p, j) pairing; out = sum_j lhsT[:,j,:].T @ rhs[:,j,:].

Engine balance: the PE is the bottleneck (~650 matmul instructions at
the HW-measured ~255-265 ns for a chained self-loading N=512 matmul),
so everything else is kept off it: the softmax rowsum is an
elementwise DVE accumulation of the 16 exp tiles plus ONE fp32
ones-matmul for the partition reduction; all PSUM evacuations are DVE
(fused scale+bias via tensor_scalar / scalar_tensor_tensor), leaving
ScalarE to run Exp only (also avoids act-table reloads). DMA rides all
three queues (sync/SP, scalar/ACT, gpsimd). A ~4.5us burst of
dependency-free warm-up matmuls at t=0 brings the PE HAM clock gate to
8/8 while the prologue DMAs stream.

(The 4-way col-group tile_position packing of the rowsum - kept behind
packed_rowsum=True - is numerically correct in CoreSim and in an
isolated HW micro-test but corrupts results at ~1.8e-1 rel err when
its packed matmuls interleave with full-array matmuls in the
production kernel, so it stays off.)

Device dataflow:
  Y[dc,q] = M2^T.T @ xqT (bf16, 128 MMs); DVE evac scale 128 + 128z
    -> yt2 fp8 [128,2,QH] x4
  per q-block of 512 (pass1/2/3):
    pass1: attT[k,q] += xt2^T.T @ yt2 (4 fp8-DR MMs per k-tile);
      PT = exp(psa/128) (ScalarE, PSUM->SBUF bf16);
      rowsum: DVE accumulates the 16 PT tiles (f32), one onesF fp32
      matmul reduces partitions -> [1,512]; per 128-q block a tiny
      [1,128]^T@[1,1] matmul transposes, feeding the DVE reciprocal
    pass2: tmpT[d,q] += xN_slice^T.T @ PT (bf16, PSUM over 16 k-tiles)
    pass3: out[q,e] += tmpT_slice^T.T @ WvT (bf16); DVE evac fuses
      out = out * (1/S) + bv -> DMA to DRAM

All weights (M2, WvT) and x layouts are SBUF-resident after the
prologue; the steady-state loop does no input DMA.

Measured on the axon TRN2 pod: ~164.8 us/rep (slope method) vs
219.1 us for the all-bf16 baseline; per-phase marginals (Y 32.9,
pass1 16.1, pass2 30.5, pass3 16.9 us) each sit at the HW
instruction floor of ~230-265 ns per N=512 matmul. rel err
1.402e-2 (gate 2e-2), matching the numpy fp8 error model.
"""

import os
import sys

for _p in ("/opt/trn_rl_repo", "/root/.axon_site/_ro/trn_rl_repo"):
    if os.path.isdir(_p) and _p not in sys.path:
        sys.path.insert(0, _p)

import numpy as np
import ml_dtypes

import concourse.bass as bass
import concourse.tile as tile
from concourse import bacc, mybir
from concourse.bass_utils import run_bass_kernel_spmd

BF16 = ml_dtypes.bfloat16
FP8 = ml_dtypes.float8_e4m3
F32 = mybir.dt.float32
CDT = mybir.dt.bfloat16
F8 = mybir.dt.float8e4
DR = mybir.MatmulPerfMode.DoubleRow

B, S, D = 4, 2048, 1024
N_CORES = 8
P = 128
DT = D // P          # 8 d-tiles (contraction)
DP = DT // 2         # 4 d-tile PAIRS for DoubleRow
KT_N = S // P        # 16 k-tiles
QH = S // 2          # 1024 queries per core
QB = 512             # q-block for phase B
NQB = QH // QB       # 2 q-blocks
QS = QB // P         # 4 q-subtiles per block

Y_SCALE = 128.0      # scale of Y as stored in fp8

_NC_CACHE = {}


def build_nc(reps: int = 1, mode: str = "full", *, dr: bool = True,
             packed_rowsum: bool = False):
    # packed_rowsum (col-group tile_position packing) measures 1.8e-1
    # rel err on real HW despite being correct in CoreSim and in an
    # isolated micro-test -- some PE weight-buffer hazard when packed
    # matmuls interleave with full-array matmuls. Keep it off.
    nc = bacc.Bacc("TRN2", target_bir_lowering=False, debug=False,
                   num_devices=N_CORES)
    Exp = mybir.ActivationFunctionType.Exp
    Copy = mybir.ActivationFunctionType.Copy

    xT8_d = nc.dram_tensor("xT8", [D, S], F8, kind="ExternalInput").ap()
    xTb_d = nc.dram_tensor("xTb", [D, QH], CDT, kind="ExternalInput").ap()
    xN_d = nc.dram_tensor("xN", [S, D], CDT, kind="ExternalInput").ap()
    m2_d = nc.dram_tensor("M2", [D, D], CDT, kind="ExternalInput").ap()
    wvT_d = nc.dram_tensor("WvT", [D, D], CDT, kind="ExternalInput").ap()
    z_d = nc.dram_tensor("z2", [P, DT], F32, kind="ExternalInput").ap()
    bv_d = nc.dram_tensor("bvb", [P, D], F32, kind="ExternalInput").ap()
    out_d = nc.dram_tensor("out", [QH, D], F32, kind="ExternalOutput").ap()

    with tile.TileContext(nc) as tc:
        with (
            tc.tile_pool(name="resident", bufs=1) as res,
            tc.tile_pool(name="pt", bufs=2) as ptpool,
            tc.tile_pool(name="tm", bufs=2) as tmpool,
            tc.tile_pool(name="osb", bufs=4) as opool,
            tc.tile_pool(name="small", bufs=4) as spool,
            tc.tile_pool(name="ps", bufs=4, space="PSUM") as psA,
            tc.tile_pool(name="ptm", bufs=3, space="PSUM") as psT,
            tc.tile_pool(name="pss", bufs=1, space="PSUM") as psS,
        ):
            # ---- resident loads (once) ----
            # Order matters for the single-shot prologue: the Y matmuls
            # need z + M2 + xTb first; xt2 (pass1 lhsT) next; xN (pass2)
            # and bv (epilogue) last. Loads alternate between the sync
            # and scalar HWDGE queues.
            z_sb = res.tile([P, DT], F32, tag="z", name="z_sb")
            nc.scalar.dma_start(z_sb[:], z_d[:, :])
            m2 = [res.tile([P, D], CDT, tag=f"m{d}", name=f"m2_{d}")
                  for d in range(DT)]
            for d in range(DT):
                (nc.sync if d % 2 else nc.scalar).dma_start(
                    m2[d][:], m2_d[d * P:(d + 1) * P, :])
            xtb = [res.tile([P, QH], CDT, tag=f"xb{d}", name=f"xb{d}")
                   for d in range(DT)]
            for half in range(2):
                cols = slice(half * 512, (half + 1) * 512)
                for d in range(DT):
                    (nc.sync if d % 2 else nc.scalar).dma_start(
                        xtb[d][:, cols], xTb_d[d * P:(d + 1) * P, cols])
            xt2 = [res.tile([P, 2, S], F8, tag=f"xt{dp}", name=f"xt{dp}")
                   for dp in range(DP)]
            # pass1 reads xt2 k-tiles in order: land the first column
            # half of every (dp, j) row before any second half, so the
            # first q-block's score matmuls can start sooner.
            for half in range(2):
                cols = slice(half * QH, (half + 1) * QH)
                for dp in range(DP):
                    for j in range(2):
                        nc.gpsimd.dma_start(
                            xt2[dp][:, j, cols],
                            xT8_d[(2 * dp + j) * P:(2 * dp + j + 1) * P,
                                  cols])
            xn = [res.tile([P, D], CDT, tag=f"xn{k}", name=f"xn{k}")
                  for k in range(KT_N)]
            bv_sb = res.tile([P, D], F32, tag="bv", name="bv_sb")
            _xnq = [nc.sync, nc.scalar, nc.gpsimd]
            for k in range(KT_N):
                _xnq[k % 3].dma_start(xn[k][:], xN_d[k * P:(k + 1) * P, :])
            nc.scalar.dma_start(bv_sb[:], bv_d[:, :])
            wv = [res.tile([P, D], CDT, tag=f"w{d}", name=f"wv_{d}")
                  for d in range(DT)]
            for d in range(DT):
                (nc.sync if d % 2 else nc.scalar).dma_start(
                    wv[d][:], wvT_d[d * P:(d + 1) * P, :])
            ones = res.tile([P, 1], CDT, tag="ones", name="ones")
            nc.vector.memset(ones[:], 1.0)
            onesF = res.tile([P, 1], F32, tag="onesF", name="onesF")
            nc.vector.memset(onesF[:], 1.0)
            one11 = res.tile([1, 1], F32, tag="one11", name="one11")
            nc.vector.memset(one11[:], 1.0)

            yt2 = [res.tile([P, 2, QH], F8, tag=f"yt{dp}", name=f"yt{dp}")
                   for dp in range(DP)]

            # PE warm-up: ~4.5us of dependency-free matmuls (memset
            # operands, no DMA deps) keep the PE busy from t=0 so the
            # HAM clock gate is at 8/8 when the first Y matmul issues;
            # the PE would otherwise idle here waiting on the prologue
            # DMAs anyway.
            warm = res.tile([P, P], CDT, tag="warm", name="warm")
            nc.vector.memset(warm[:], 0.5)
            wps = psS.tile([P, QB], F32, tag="pss", name="wps")
            for _w in range(40):
                nc.tensor.matmul(wps[0:1, 0:P], lhsT=warm[:, 0:1],
                                 rhs=warm[:, :], start=True, stop=True,
                                 skip_group_check=True)

            def y_phase():
                for sb in range(QH // 512):
                    for dc in range(DT):
                        ps = psA.tile([P, 512], F32, tag="ps", name="ps")
                        for d in range(DT):
                            nc.tensor.matmul(
                                ps[:],
                                lhsT=m2[d][:, dc * P:(dc + 1) * P],
                                rhs=xtb[d][:, sb * 512:(sb + 1) * 512],
                                start=(d == 0), stop=(d == DT - 1))
                        # yt = 128*Y + z2 in fp8 (z2 host-scaled by 128).
                        # On DVE so ScalarE stays Exp-only (no act-table
                        # reloads between functions).
                        nc.vector.tensor_scalar(
                            out=yt2[dc // 2][:, dc % 2,
                                             sb * 512:(sb + 1) * 512],
                            in0=ps[:], scalar1=Y_SCALE,
                            scalar2=z_sb[:, dc:dc + 1],
                            op0=mybir.AluOpType.mult,
                            op1=mybir.AluOpType.add)

            def pass1(qb):
                # ---- pass 1: scores (fp8 DR), exp, row sums ----
                srow_ps = psS.tile([P, QB], F32, tag="pss",
                                   name="srow_ps")
                pts = []
                for k in range(KT_N):
                    psa = psA.tile([P, QB], F32, tag="ps", name="psa")
                    for dp in range(DP):
                        nc.tensor.matmul(
                            psa[:],
                            lhsT=xt2[dp][:, :, k * P:(k + 1) * P],
                            rhs=yt2[dp][:, :, qb * QB:(qb + 1) * QB],
                            start=(dp == 0), stop=(dp == DP - 1),
                            perf_mode=DR)
                    pt_sb = ptpool.tile([P, QB], CDT, tag=f"pt{k}",
                                        name=f"pt_sb{k}")
                    # PT = exp(psa/128)
                    nc.scalar.activation(pt_sb[:], psa[:], Exp,
                                         scale=1.0 / Y_SCALE)
                    pts.append(pt_sb)
                if packed_rowsum:
                    # rowsum: 4 col-group-packed chains of 4 k-tiles
                    # each; group g accumulates into partition 32*g.
                    for g in range(4):
                        for i in range(4):
                            k = 4 * g + i
                            nc.tensor.matmul(
                                srow_ps[32 * g:32 * g + 1, :],
                                lhsT=ones[:], rhs=pts[k][:],
                                start=(i == 0), stop=(i == 3),
                                tile_position=(0, 32 * g),
                                skip_group_check=True)
                    srow_sb = spool.tile([P, QB], F32, tag="srow",
                                         name="srow_sb")
                    for g in range(4):
                        nc.vector.tensor_copy(
                            out=srow_sb[32 * g:32 * g + 1, :],
                            in_=srow_ps[32 * g:32 * g + 1, :])
                    recs = []
                    for qs in range(QS):
                        scol_ps = psA.tile([P, 1], F32, tag="ps",
                                           name="scol_ps")
                        # sum the 4 group partials while transposing:
                        # 4 accumulating K=1 matmuls at row groups 32g
                        for g in range(4):
                            nc.tensor.matmul(
                                scol_ps[:],
                                lhsT=srow_sb[32 * g:32 * g + 1,
                                             qs * P:(qs + 1) * P],
                                rhs=onesF[32 * g:32 * g + 1, :],
                                start=(g == 0), stop=(g == 3),
                                tile_position=(32 * g, 0),
                                skip_group_check=True)
                        rec = spool.tile([P, 1], F32, tag="rec",
                                         name="rec")
                        nc.vector.reciprocal(rec[:], scol_ps[:])
                        recs.append(rec)
                else:
                    # Accumulate the 16 PT tiles elementwise on DVE
                    # (f32), then one fp32 ones-matmul reduces across
                    # partitions. Keeps 15 N=512 matmuls off the PE.
                    acc = spool.tile([P, QB], F32, tag="acc", name="acc")
                    nc.vector.tensor_add(acc[:], pts[0][:], pts[1][:])
                    for k in range(2, KT_N):
                        nc.vector.tensor_add(acc[:], acc[:], pts[k][:])
                    nc.tensor.matmul(
                        srow_ps[0:1, :], lhsT=onesF[:], rhs=acc[:],
                        start=True, stop=True)
                    srow_sb = spool.tile([1, QB], F32, tag="srow",
                                         name="srow_sb")
                    nc.vector.tensor_copy(out=srow_sb[:],
                                          in_=srow_ps[0:1, :])
                    recs = []
                    for qs in range(QS):
                        scol_ps = psA.tile([P, 1], F32, tag="ps",
                                           name="scol_ps")
                        nc.tensor.matmul(
                            scol_ps[:],
                            lhsT=srow_sb[0:1, qs * P:(qs + 1) * P],
                            rhs=one11[:], start=True, stop=True)
                        rec = spool.tile([P, 1], F32, tag="rec",
                                         name="rec")
                        nc.vector.reciprocal(rec[:], scol_ps[:])
                        recs.append(rec)
                return pts, recs

            def pass2(pts):
                # ---- pass 2: tmpT[d, q] = sum_k x_k^T P^T (bf16) ----
                tms = []
                for dt_i in range(DT):
                    pst = psT.tile([P, 512], F32, tag="ptm", name="pst")
                    for k in range(KT_N):
                        nc.tensor.matmul(
                            pst[:],
                            lhsT=xn[k][:, dt_i * P:(dt_i + 1) * P],
                            rhs=pts[k][:],
                            start=(k == 0), stop=(k == KT_N - 1))
                    tm = tmpool.tile([P, 512], CDT, tag=f"tm{dt_i}",
                                     name=f"tm{dt_i}")
                    nc.vector.tensor_copy(out=tm[:], in_=pst[:])
                    tms.append(tm)
                return tms

            def pass3(qb, tms, recs):
                # ---- pass 3: out[q, e] = tmpT^T @ WvT, scaled + bv ----
                for qs in range(QS):
                    for eb in range(2):
                        pso = psA.tile([P, 512], F32, tag="ps",
                                       name="pso")
                        for dt_i in range(DT):
                            nc.tensor.matmul(
                                pso[:],
                                lhsT=tms[dt_i][:, qs * P:(qs + 1) * P],
                                rhs=wv[dt_i][:, eb * 512:(eb + 1) * 512],
                                start=(dt_i == 0), stop=(dt_i == DT - 1))
                        osb = opool.tile([P, 512], F32, tag="osb",
                                         name="osb")
                        # out = pso * (1/S) + bv, fused on DVE
                        nc.vector.scalar_tensor_tensor(
                            out=osb[:], in0=pso[:], scalar=recs[qs][:],
                            in1=bv_sb[:, eb * 512:(eb + 1) * 512],
                            op0=mybir.AluOpType.mult,
                            op1=mybir.AluOpType.add)
                        row = qb * QB + qs * P
                        (nc.sync if eb else nc.gpsimd).dma_start(
                            out_d[row:row + P, eb * 512:(eb + 1) * 512],
                            osb[:])

            a_reps = reps if mode in ("full", "A") else 1
            b_reps = reps if mode in ("full", "B") else 1
            for _i in range(a_reps):
                y_phase()
            for _i in range(b_reps):
                for qb in range(NQB):
                    pts, recs = pass1(qb)
                    if mode == "P1" and qb == 0:
                        for _j in range(reps - 1):
                            pts, recs = pass1(0)
                    tms = pass2(pts)
                    if mode == "P2" and qb == 0:
                        for _j in range(reps - 1):
                            tms = pass2(pts)
                    pass3(qb, tms, recs)
                    if mode == "P3" and qb == 0:
                        for _j in range(reps - 1):
                            pass3(0, tms, recs)
            if mode == "A":
                nc.gpsimd.dma_start(out_d[0:P, 0:8], yt2[0][:, 0, 0:8])
    nc.compile()
    return nc


def _get_nc(reps: int = 1, mode: str = "full"):
    key = (reps, mode)
    if key not in _NC_CACHE:
        _NC_CACHE[key] = build_nc(reps, mode)
    return _NC_CACHE[key]


def make_in_maps(x, Wq, bq, Wk, bk, Wv, bv):
    inv = np.float64(1.0 / np.sqrt(D))
    M2 = Wq.T.astype(np.float64) @ Wk.astype(np.float64) * inv
    z = Wk.T.astype(np.float64) @ bq.astype(np.float64) * inv
    m2b = np.ascontiguousarray(M2.astype(np.float32)).astype(BF16)
    wvT = np.ascontiguousarray(Wv.T).astype(BF16)
    z2 = np.ascontiguousarray(
        (z * Y_SCALE).astype(np.float32).reshape(DT, P).T).astype(np.float32)
    bvb = np.ascontiguousarray(np.broadcast_to(bv, (P, D))).astype(np.float32)
    in_maps = []
    for c in range(N_CORES):
        b, h = divmod(c, 2)
        # rotate the sequence axis so this core's query half is always
        # columns/rows 0:QH -- attention is permutation-invariant along
        # the key axis as long as xT (pass1) and xN (pass2) agree.
        xr = np.roll(x[b], -h * QH, axis=0)
        xT = np.ascontiguousarray(xr.T)
        xT8 = xT.astype(FP8)
        xTb = np.ascontiguousarray(xT[:, :QH]).astype(BF16)
        xN = np.ascontiguousarray(xr).astype(BF16)
        in_maps.append({
            "xT8": xT8, "xTb": xTb, "xN": xN,
            "M2": m2b, "WvT": wvT,
            "z2": z2, "bvb": bvb,
        })
    return in_maps


def kernel(x, Wq, bq, Wk, bk, Wv, bv):
    x = np.asarray(x, np.float32)
    in_maps = make_in_maps(x, np.asarray(Wq, np.float32),
                           np.asarray(bq, np.float32),
                           np.asarray(Wk, np.float32),
                           np.asarray(bk, np.float32),
                           np.asarray(Wv, np.float32),
                           np.asarray(bv, np.float32))
    nc = _get_nc()
    res = run_bass_kernel_spmd(nc, in_maps, core_ids=list(range(N_CORES)))
    out = np.empty((B, S, D), np.float32)
    for c in range(N_CORES):
        b, h = divmod(c, 2)
        out[b, h * QH:(h + 1) * QH, :] = res.results[c]["out"]
    return out


# revision 34
# speedup vs baseline: 1.1276x; 1.1276x over previous
"""Fused single-head attention kernel for 8 TRN2 NeuronCores.

Problem: B=4, S=2048, D=1024 attention:
    Q = x @ Wq.T + bq; K = x @ Wk.T + bk; V = x @ Wv.T + bv
    out = softmax(Q K^T / sqrt(D)) @ V

Sharding (no cross-core traffic): core c handles batch b = c//2 and
query half h = c%2 (1024 queries).

Algebraic refactoring: the K/V projections are folded away:
  logits[k,q] = x_k . (M2^T x_q + z)   (M2 = Wq^T Wk / sqrt(D),
                                        z = Wk^T bq / sqrt(D);
  per-query terms drop out of the softmax). out = ((P x) Wv^T)/rowsum(P)
  + bv, so V is never materialized.

The S x QH logits matmul (pass1, the largest single score-path item)
runs in fp8-e4m3 with MatmulPerfMode.DoubleRow (K=256 per matmul,
2 fp8 weights per PE cell), roughly halving its TensorE time. Y itself
is computed in bf16 (fp8 M2/x there pushes rel-err past the 2e-2 gate:
measured 2.1e-2 full-fp8 vs 1.4e-2 with bf16 Y; bf16 baseline 2.3e-3).
The output path (pass2: x^T P, pass3: (.)Wv^T) stays bf16.

fp8 scale management (e4m3 min normal 2^-6): Y has rms ~0.01, so the
Y evacuation scales by 128 (-> rms ~1.3 in fp8; bias z is host-scaled
by 128 and added in the same activation, before quantization). pass1
PSUM then holds 128*logits and the exp evac uses scale 1/128.

DoubleRow data layout: 3D tiles [128, 2, N]; element (p, j, n) is
contraction index (2*dp+j)*128+p for d-tile-pair dp. Both operands use
the same (p, j) pairing; out = sum_j lhsT[:,j,:].T @ rhs[:,j,:].

Engine balance: the PE is the bottleneck (~650 matmul instructions at
the HW-measured ~255-265 ns for a chained self-loading N=512 matmul),
so everything else is kept off it: the softmax rowsum is an
elementwise DVE accumulation of the 16 exp tiles plus ONE fp32
ones-matmul for the partition reduction; all PSUM evacuations are DVE
(fused scale+bias via tensor_scalar / scalar_tensor_tensor), leaving
ScalarE to run Exp only (also avoids act-table reloads). DMA rides all
three queues (sync/SP, scalar/ACT, gpsimd). A ~4.5us burst of
dependency-free warm-up matmuls at t=0 brings the PE HAM clock gate to
8/8 while the prologue DMAs stream.

(The 4-way col-group tile_position packing of the rowsum - kept behind
packed_rowsum=True - is numerically correct in CoreSim and in an
isolated HW micro-test but corrupts results at ~1.8e-1 rel err when
its packed matmuls interleave with full-array matmuls in the
production kernel, so it stays off.)

Device dataflow:
  Y[dc,q] = M2^T.T @ xqT (bf16, 128 MMs); DVE evac scale 128 + 128z
    -> yt2 fp8 [128,2,QH] x4
  per q-block of 512 (pass1/2/3):
    pass1: attT[k,q] += xt2^T.T @ yt2 (4 fp8-DR MMs per k-tile);
      PT = exp(psa/128) (ScalarE, PSUM->SBUF bf16);
      rowsum: DVE accumulates the 16 PT tiles (f32), one onesF fp32
      matmul reduces partitions -> [1,512]; per 128-q block a tiny
      [1,128]^T@[1,1] matmul transposes, feeding the DVE reciprocal
    pass2: tmpT[d,q] += xN_slice^T.T @ PT (bf16, PSUM over 16 k-tiles)
    pass3: out[q,e] += tmpT_slice^T.T @ WvT (bf16); DVE evac fuses
      out = out * (1/S) + bv -> DMA to DRAM

All weights (M2, WvT) and x layouts are SBUF-resident after the
prologue; the steady-state loop does no input DMA.

Measured on the axon TRN2 pod: ~164.8 us/rep (slope method) vs
219.1 us for the all-bf16 baseline; per-phase marginals (Y 32.9,
pass1 16.1, pass2 30.5, pass3 16.9 us) each sit at the HW
instruction floor of ~230-265 ns per N=512 matmul. rel err
1.402e-2 (gate 2e-2), matching the numpy fp8 error model.
"""

import os
import sys

for _p in ("/opt/trn_rl_repo", "/root/.axon_site/_ro/trn_rl_repo"):
    if os.path.isdir(_p) and _p not in sys.path:
        sys.path.insert(0, _p)

import numpy as np
import ml_dtypes

import concourse.bass as bass
import concourse.tile as tile
from concourse import bacc, mybir
from concourse.bass_utils import run_bass_kernel_spmd

BF16 = ml_dtypes.bfloat16
FP8 = ml_dtypes.float8_e4m3
F32 = mybir.dt.float32
CDT = mybir.dt.bfloat16
F8 = mybir.dt.float8e4
DR = mybir.MatmulPerfMode.DoubleRow

B, S, D = 4, 2048, 1024
N_CORES = 8
P = 128
DT = D // P          # 8 d-tiles (contraction)
DP = DT // 2         # 4 d-tile PAIRS for DoubleRow
KT_N = S // P        # 16 k-tiles
QH = S // 2          # 1024 queries per core
QB = 512             # q-block for phase B
NQB = QH // QB       # 2 q-blocks
QS = QB // P         # 4 q-subtiles per block

Y_SCALE = 128.0      # scale of Y as stored in fp8

_NC_CACHE = {}


def build_nc(reps: int = 1, mode: str = "full", *, dr: bool = True,
             packed_rowsum: bool = False):
    # packed_rowsum (col-group tile_position packing) measures 1.8e-1
    # rel err on real HW despite being correct in CoreSim and in an
    # isolated micro-test -- some PE weight-buffer hazard when packed
    # matmuls interleave with full-array matmuls. Keep it off.
    nc = bacc.Bacc("TRN2", target_bir_lowering=False, debug=False,
                   num_devices=N_CORES)
    Exp = mybir.ActivationFunctionType.Exp
    Copy = mybir.ActivationFunctionType.Copy

    xT8_d = nc.dram_tensor("xT8", [D, S], F8, kind="ExternalInput").ap()
    xTb_d = nc.dram_tensor("xTb", [D, QH], CDT, kind="ExternalInput").ap()
    xN_d = nc.dram_tensor("xN", [S, D], CDT, kind="ExternalInput").ap()
    m2_d = nc.dram_tensor("M2", [D, D], CDT, kind="ExternalInput").ap()
    wvT_d = nc.dram_tensor("WvT", [D, D], CDT, kind="ExternalInput").ap()
    z_d = nc.dram_tensor("z2", [P, DT], F32, kind="ExternalInput").ap()
    bv_d = nc.dram_tensor("bvb", [P, D], F32, kind="ExternalInput").ap()
    out_d = nc.dram_tensor("out", [QH, D], F32, kind="ExternalOutput").ap()

    with tile.TileContext(nc) as tc:
        with (
            tc.tile_pool(name="resident", bufs=1) as res,
            tc.tile_pool(name="pt", bufs=2) as ptpool,
            tc.tile_pool(name="tm", bufs=2) as tmpool,
            tc.tile_pool(name="osb", bufs=4) as opool,
            tc.tile_pool(name="small", bufs=4) as spool,
            tc.tile_pool(name="ps", bufs=4, space="PSUM") as psA,
            tc.tile_pool(name="ptm", bufs=3, space="PSUM") as psT,
            tc.tile_pool(name="pss", bufs=1, space="PSUM") as psS,
        ):
            # ---- resident loads (once) ----
            # Order matters for the single-shot prologue: the Y matmuls
            # need z + M2 + xTb first; xt2 (pass1 lhsT) next; xN (pass2)
            # and bv (epilogue) last. Loads alternate between the sync
            # and scalar HWDGE queues.
            z_sb = res.tile([P, DT], F32, tag="z", name="z_sb")
            nc.scalar.dma_start(z_sb[:], z_d[:, :])
            m2 = [res.tile([P, D], CDT, tag=f"m{d}", name=f"m2_{d}")
                  for d in range(DT)]
            for d in range(DT):
                (nc.sync if d % 2 else nc.scalar).dma_start(
                    m2[d][:], m2_d[d * P:(d + 1) * P, :])
            xtb = [res.tile([P, QH], CDT, tag=f"xb{d}", name=f"xb{d}")
                   for d in range(DT)]
            for half in range(2):
                cols = slice(half * 512, (half + 1) * 512)
                for d in range(DT):
                    (nc.sync if d % 2 else nc.scalar).dma_start(
                        xtb[d][:, cols], xTb_d[d * P:(d + 1) * P, cols])
            xt2 = [res.tile([P, 2, S], F8, tag=f"xt{dp}", name=f"xt{dp}")
                   for dp in range(DP)]
            # pass1 reads xt2 k-tiles in order: land the first column
            # half of every (dp, j) row before any second half, so the
            # first q-block's score matmuls can start sooner.
            for half in range(2):
                cols = slice(half * QH, (half + 1) * QH)
                for dp in range(DP):
                    for j in range(2):
                        nc.gpsimd.dma_start(
                            xt2[dp][:, j, cols],
                            xT8_d[(2 * dp + j) * P:(2 * dp + j + 1) * P,
                                  cols])
            xn = [res.tile([P, D], CDT, tag=f"xn{k}", name=f"xn{k}")
                  for k in range(KT_N)]
            bv_sb = res.tile([P, D], F32, tag="bv", name="bv_sb")
            _xnq = [nc.sync, nc.scalar, nc.gpsimd]
            for k in range(KT_N):
                _xnq[k % 3].dma_start(xn[k][:], xN_d[k * P:(k + 1) * P, :])
            nc.scalar.dma_start(bv_sb[:], bv_d[:, :])
            wv = [res.tile([P, D], CDT, tag=f"w{d}", name=f"wv_{d}")
                  for d in range(DT)]
            for d in range(DT):
                (nc.sync if d % 2 else nc.scalar).dma_start(
                    wv[d][:], wvT_d[d * P:(d + 1) * P, :])
            ones = res.tile([P, 1], CDT, tag="ones", name="ones")
            nc.vector.memset(ones[:], 1.0)
            onesF = res.tile([P, 1], F32, tag="onesF", name="onesF")
            nc.vector.memset(onesF[:], 1.0)
            one11 = res.tile([1, 1], F32, tag="one11", name="one11")
            nc.vector.memset(one11[:], 1.0)

            yt2 = [res.tile([P, 2, QH], F8, tag=f"yt{dp}", name=f"yt{dp}")
                   for dp in range(DP)]

            # PE warm-up: ~4.5us of dependency-free matmuls (memset
            # operands, no DMA deps) keep the PE busy from t=0 so the
            # HAM clock gate is at 8/8 when the first Y matmul issues;
            # the PE would otherwise idle here waiting on the prologue
            # DMAs anyway.
            warm = res.tile([P, P], CDT, tag="warm", name="warm")
            nc.vector.memset(warm[:], 0.5)
            wps = psS.tile([P, QB], F32, tag="pss", name="wps")
            for _w in range(40):
                nc.tensor.matmul(wps[0:1, 0:P], lhsT=warm[:, 0:1],
                                 rhs=warm[:, :], start=True, stop=True,
                                 skip_group_check=True)

            def y_phase():
                for sb in range(QH // 512):
                    for dc in range(DT):
                        ps = psA.tile([P, 512], F32, tag="ps", name="ps")
                        for d in range(DT):
                            nc.tensor.matmul(
                                ps[:],
                                lhsT=m2[d][:, dc * P:(dc + 1) * P],
                                rhs=xtb[d][:, sb * 512:(sb + 1) * 512],
                                start=(d == 0), stop=(d == DT - 1))
                        # yt = 128*Y + z2 in fp8 (z2 host-scaled by 128).
                        # On DVE so ScalarE stays Exp-only (no act-table
                        # reloads between functions).
                        nc.vector.tensor_scalar(
                            out=yt2[dc // 2][:, dc % 2,
                                             sb * 512:(sb + 1) * 512],
                            in0=ps[:], scalar1=Y_SCALE,
                            scalar2=z_sb[:, dc:dc + 1],
                            op0=mybir.AluOpType.mult,
                            op1=mybir.AluOpType.add)

            def pass1(qb):
                # ---- pass 1: scores (fp8 DR), exp, row sums ----
                srow_ps = psS.tile([P, QB], F32, tag="pss",
                                   name="srow_ps")
                pts = []
                for k in range(KT_N):
                    psa = psA.tile([P, QB], F32, tag="ps", name="psa")
                    for dp in range(DP):
                        nc.tensor.matmul(
                            psa[:],
                            lhsT=xt2[dp][:, :, k * P:(k + 1) * P],
                            rhs=yt2[dp][:, :, qb * QB:(qb + 1) * QB],
                            start=(dp == 0), stop=(dp == DP - 1),
                            perf_mode=DR)
                    pt_sb = ptpool.tile([P, QB], CDT, tag=f"pt{k}",
                                        name=f"pt_sb{k}")
                    # PT = exp(psa/128)
                    nc.scalar.activation(pt_sb[:], psa[:], Exp,
                                         scale=1.0 / Y_SCALE)
                    pts.append(pt_sb)
                accb = None
                if packed_rowsum:
                    # rowsum: 4 col-group-packed chains of 4 k-tiles
                    # each; group g accumulates into partition 32*g.
                    for g in range(4):
                        for i in range(4):
                            k = 4 * g + i
                            nc.tensor.matmul(
                                srow_ps[32 * g:32 * g + 1, :],
                                lhsT=ones[:], rhs=pts[k][:],
                                start=(i == 0), stop=(i == 3),
                                tile_position=(0, 32 * g),
                                skip_group_check=True)
                    srow_sb = spool.tile([P, QB], F32, tag="srow",
                                         name="srow_sb")
                    for g in range(4):
                        nc.vector.tensor_copy(
                            out=srow_sb[32 * g:32 * g + 1, :],
                            in_=srow_ps[32 * g:32 * g + 1, :])
                    recs = []
                    for qs in range(QS):
                        scol_ps = psA.tile([P, 1], F32, tag="ps",
                                           name="scol_ps")
                        # sum the 4 group partials while transposing:
                        # 4 accumulating K=1 matmuls at row groups 32g
                        for g in range(4):
                            nc.tensor.matmul(
                                scol_ps[:],
                                lhsT=srow_sb[32 * g:32 * g + 1,
                                             qs * P:(qs + 1) * P],
                                rhs=onesF[32 * g:32 * g + 1, :],
                                start=(g == 0), stop=(g == 3),
                                tile_position=(32 * g, 0),
                                skip_group_check=True)
                        rec = spool.tile([P, 1], F32, tag="rec",
                                         name="rec")
                        nc.vector.reciprocal(rec[:], scol_ps[:])
                        recs.append(rec)
                else:
                    # Accumulate the 16 PT tiles elementwise on DVE
                    # (f32; final add emits a bf16 copy so the
                    # partition-reduce matmul runs at bf16 rate).
                    # Keeps 15 N=512 matmuls off the PE. The matmul
                    # part of the reduction is emitted later, AFTER
                    # pass2 (rowsum_reduce), so the PE never waits on
                    # this DVE chain.
                    acc = spool.tile([P, QB], F32, tag="acc", name="acc")
                    nc.vector.tensor_add(acc[:], pts[0][:], pts[1][:])
                    for k in range(2, KT_N - 1):
                        nc.vector.tensor_add(acc[:], acc[:], pts[k][:])
                    accb = spool.tile([P, QB], CDT, tag="accb",
                                      name="accb")
                    nc.vector.tensor_add(accb[:], acc[:],
                                         pts[KT_N - 1][:])
                    recs = None
                return pts, accb, recs

            def rowsum_reduce(accb):
                # partition-reduce the DVE-accumulated exp sums (bf16
                # ones-matmul), transpose per 128-q block, reciprocal.
                srow_ps = psS.tile([P, QB], F32, tag="pss",
                                   name="srow_ps")
                nc.tensor.matmul(
                    srow_ps[0:1, :], lhsT=ones[:], rhs=accb[:],
                    start=True, stop=True)
                srow_sb = spool.tile([1, QB], F32, tag="srow",
                                     name="srow_sb")
                nc.vector.tensor_copy(out=srow_sb[:],
                                      in_=srow_ps[0:1, :])
                recs = []
                for qs in range(QS):
                    scol_ps = psA.tile([P, 1], F32, tag="ps",
                                       name="scol_ps")
                    nc.tensor.matmul(
                        scol_ps[:],
                        lhsT=srow_sb[0:1, qs * P:(qs + 1) * P],
                        rhs=one11[:], start=True, stop=True)
                    rec = spool.tile([P, 1], F32, tag="rec",
                                     name="rec")
                    nc.vector.reciprocal(rec[:], scol_ps[:])
                    recs.append(rec)
                return recs

            def pass2(pts):
                # ---- pass 2: tmpT[d, q] = sum_k x_k^T P^T (bf16) ----
                tms = []
                for dt_i in range(DT):
                    pst = psT.tile([P, 512], F32, tag="ptm", name="pst")
                    for k in range(KT_N):
                        nc.tensor.matmul(
                            pst[:],
                            lhsT=xn[k][:, dt_i * P:(dt_i + 1) * P],
                            rhs=pts[k][:],
                            start=(k == 0), stop=(k == KT_N - 1))
                    tm = tmpool.tile([P, 512], CDT, tag=f"tm{dt_i}",
                                     name=f"tm{dt_i}")
                    nc.vector.tensor_copy(out=tm[:], in_=pst[:])
                    tms.append(tm)
                return tms

            def pass3(qb, tms, recs):
                # ---- pass 3: out[q, e] = tmpT^T @ WvT, scaled + bv ----
                for qs in range(QS):
                    for eb in range(2):
                        pso = psA.tile([P, 512], F32, tag="ps",
                                       name="pso")
                        for dt_i in range(DT):
                            nc.tensor.matmul(
                                pso[:],
                                lhsT=tms[dt_i][:, qs * P:(qs + 1) * P],
                                rhs=wv[dt_i][:, eb * 512:(eb + 1) * 512],
                                start=(dt_i == 0), stop=(dt_i == DT - 1))
                        osb = opool.tile([P, 512], F32, tag="osb",
                                         name="osb")
                        # out = pso * (1/S) + bv, fused on DVE
                        nc.vector.scalar_tensor_tensor(
                            out=osb[:], in0=pso[:], scalar=recs[qs][:],
                            in1=bv_sb[:, eb * 512:(eb + 1) * 512],
                            op0=mybir.AluOpType.mult,
                            op1=mybir.AluOpType.add)
                        row = qb * QB + qs * P
                        (nc.sync if eb else nc.gpsimd).dma_start(
                            out_d[row:row + P, eb * 512:(eb + 1) * 512],
                            osb[:])

            a_reps = reps if mode in ("full", "A") else 1
            b_reps = reps if mode in ("full", "B") else 1
            for _i in range(a_reps):
                y_phase()
            for _i in range(b_reps):
                for qb in range(NQB):
                    pts, accb, recs = pass1(qb)
                    if mode == "P1" and qb == 0:
                        for _j in range(reps - 1):
                            pts, accb, recs = pass1(0)
                    tms = pass2(pts)
                    if mode == "P2" and qb == 0:
                        for _j in range(reps - 1):
                            tms = pass2(pts)
                    if recs is None:
                        recs = rowsum_reduce(accb)
                    pass3(qb, tms, recs)
                    if mode == "P3" and qb == 0:
                        for _j in range(reps - 1):
                            pass3(0, tms, recs)
            if mode == "A":
                nc.gpsimd.dma_start(out_d[0:P, 0:8], yt2[0][:, 0, 0:8])
    nc.compile()
    return nc


def _get_nc(reps: int = 1, mode: str = "full"):
    key = (reps, mode)
    if key not in _NC_CACHE:
        _NC_CACHE[key] = build_nc(reps, mode)
    return _NC_CACHE[key]


def make_in_maps(x, Wq, bq, Wk, bk, Wv, bv):
    inv = np.float64(1.0 / np.sqrt(D))
    M2 = Wq.T.astype(np.float64) @ Wk.astype(np.float64) * inv
    z = Wk.T.astype(np.float64) @ bq.astype(np.float64) * inv
    m2b = np.ascontiguousarray(M2.astype(np.float32)).astype(BF16)
    wvT = np.ascontiguousarray(Wv.T).astype(BF16)
    z2 = np.ascontiguousarray(
        (z * Y_SCALE).astype(np.float32).reshape(DT, P).T).astype(np.float32)
    bvb = np.ascontiguousarray(np.broadcast_to(bv, (P, D))).astype(np.float32)
    in_maps = []
    for c in range(N_CORES):
        b, h = divmod(c, 2)
        # rotate the sequence axis so this core's query half is always
        # columns/rows 0:QH -- attention is permutation-invariant along
        # the key axis as long as xT (pass1) and xN (pass2) agree.
        xr = np.roll(x[b], -h * QH, axis=0)
        xT = np.ascontiguousarray(xr.T)
        xT8 = xT.astype(FP8)
        xTb = np.ascontiguousarray(xT[:, :QH]).astype(BF16)
        xN = np.ascontiguousarray(xr).astype(BF16)
        in_maps.append({
            "xT8": xT8, "xTb": xTb, "xN": xN,
            "M2": m2b, "WvT": wvT,
            "z2": z2, "bvb": bvb,
        })
    return in_maps


def kernel(x, Wq, bq, Wk, bk, Wv, bv):
    x = np.asarray(x, np.float32)
    in_maps = make_in_maps(x, np.asarray(Wq, np.float32),
                           np.asarray(bq, np.float32),
                           np.asarray(Wk, np.float32),
                           np.asarray(bk, np.float32),
                           np.asarray(Wv, np.float32),
                           np.asarray(bv, np.float32))
    nc = _get_nc()
    res = run_bass_kernel_spmd(nc, in_maps, core_ids=list(range(N_CORES)))
    out = np.empty((B, S, D), np.float32)
    for c in range(N_CORES):
        b, h = divmod(c, 2)
        out[b, h * QH:(h + 1) * QH, :] = res.results[c]["out"]
    return out


# revision 35
# speedup vs baseline: 1.1284x; 1.0008x over previous
"""Fused single-head attention kernel for 8 TRN2 NeuronCores.

Problem: B=4, S=2048, D=1024 attention:
    Q = x @ Wq.T + bq; K = x @ Wk.T + bk; V = x @ Wv.T + bv
    out = softmax(Q K^T / sqrt(D)) @ V

Sharding (no cross-core traffic): core c handles batch b = c//2 and
query half h = c%2 (1024 queries).

Algebraic refactoring: the K/V projections are folded away:
  logits[k,q] = x_k . (M2^T x_q + z)   (M2 = Wq^T Wk / sqrt(D),
                                        z = Wk^T bq / sqrt(D);
  per-query terms drop out of the softmax). out = ((P x) Wv^T)/rowsum(P)
  + bv, so V is never materialized.

The S x QH logits matmul (pass1, the largest single score-path item)
runs in fp8-e4m3 with MatmulPerfMode.DoubleRow (K=256 per matmul,
2 fp8 weights per PE cell), roughly halving its TensorE time. Y itself
is computed in bf16 (fp8 M2/x there pushes rel-err past the 2e-2 gate:
measured 2.1e-2 full-fp8 vs 1.4e-2 with bf16 Y; bf16 baseline 2.3e-3).
The output path (pass2: x^T P, pass3: (.)Wv^T) stays bf16.

fp8 scale management (e4m3 min normal 2^-6): Y has rms ~0.01, so the
Y evacuation scales by 128 (-> rms ~1.3 in fp8; bias z is host-scaled
by 128 and added in the same activation, before quantization). pass1
PSUM then holds 128*logits and the exp evac uses scale 1/128.

DoubleRow data layout: 3D tiles [128, 2, N]; element (p, j, n) is
contraction index (2*dp+j)*128+p for d-tile-pair dp. Both operands use
the same (p, j) pairing; out = sum_j lhsT[:,j,:].T @ rhs[:,j,:].

Engine balance: the PE is the bottleneck (~650 matmul instructions at
the HW-measured ~255-265 ns for a chained self-loading N=512 matmul),
so everything else is kept off it: the softmax rowsum is an
elementwise DVE accumulation of the 16 exp tiles plus ONE fp32
ones-matmul for the partition reduction; all PSUM evacuations are DVE
(fused scale+bias via tensor_scalar / scalar_tensor_tensor), leaving
ScalarE to run Exp only (also avoids act-table reloads). DMA rides all
three queues (sync/SP, scalar/ACT, gpsimd). A ~4.5us burst of
dependency-free warm-up matmuls at t=0 brings the PE HAM clock gate to
8/8 while the prologue DMAs stream.

(The 4-way col-group tile_position packing of the rowsum - kept behind
packed_rowsum=True - is numerically correct in CoreSim and in an
isolated HW micro-test but corrupts results at ~1.8e-1 rel err when
its packed matmuls interleave with full-array matmuls in the
production kernel, so it stays off.)

Device dataflow:
  Y[dc,q] = M2^T.T @ xqT (bf16, 128 MMs); DVE evac scale 128 + 128z
    -> yt2 fp8 [128,2,QH] x4
  per q-block of 512 (pass1/2/3):
    pass1: attT[k,q] += xt2^T.T @ yt2 (4 fp8-DR MMs per k-tile);
      PT = exp(psa/128) (ScalarE, PSUM->SBUF bf16);
      rowsum: DVE accumulates the 16 PT tiles (f32), one onesF fp32
      matmul reduces partitions -> [1,512]; per 128-q block a tiny
      [1,128]^T@[1,1] matmul transposes, feeding the DVE reciprocal
    pass2: tmpT[d,q] += xN_slice^T.T @ PT (bf16, PSUM over 16 k-tiles)
    pass3: out[q,e] += tmpT_slice^T.T @ WvT (bf16); DVE evac fuses
      out = out * (1/S) + bv -> DMA to DRAM

All weights (M2, WvT) and x layouts are SBUF-resident after the
prologue; the steady-state loop does no input DMA.

Measured on the axon TRN2 pod: ~164.8 us/rep (slope method) vs
219.1 us for the all-bf16 baseline; per-phase marginals (Y 32.9,
pass1 16.1, pass2 30.5, pass3 16.9 us) each sit at the HW
instruction floor of ~230-265 ns per N=512 matmul. rel err
1.402e-2 (gate 2e-2), matching the numpy fp8 error model.
"""

import os
import sys

for _p in ("/opt/trn_rl_repo", "/root/.axon_site/_ro/trn_rl_repo"):
    if os.path.isdir(_p) and _p not in sys.path:
        sys.path.insert(0, _p)

import numpy as np
import ml_dtypes

import concourse.bass as bass
import concourse.tile as tile
from concourse import bacc, mybir
from concourse.bass_utils import run_bass_kernel_spmd

BF16 = ml_dtypes.bfloat16
FP8 = ml_dtypes.float8_e4m3
F32 = mybir.dt.float32
CDT = mybir.dt.bfloat16
F8 = mybir.dt.float8e4
DR = mybir.MatmulPerfMode.DoubleRow

B, S, D = 4, 2048, 1024
N_CORES = 8
P = 128
DT = D // P          # 8 d-tiles (contraction)
DP = DT // 2         # 4 d-tile PAIRS for DoubleRow
KT_N = S // P        # 16 k-tiles
QH = S // 2          # 1024 queries per core
QB = 512             # q-block for phase B
NQB = QH // QB       # 2 q-blocks
QS = QB // P         # 4 q-subtiles per block

Y_SCALE = 128.0      # scale of Y as stored in fp8

_NC_CACHE = {}


def build_nc(reps: int = 1, mode: str = "full", *, dr: bool = True,
             packed_rowsum: bool = False):
    # packed_rowsum (col-group tile_position packing) measures 1.8e-1
    # rel err on real HW despite being correct in CoreSim and in an
    # isolated micro-test -- some PE weight-buffer hazard when packed
    # matmuls interleave with full-array matmuls. Keep it off.
    nc = bacc.Bacc("TRN2", target_bir_lowering=False, debug=False,
                   num_devices=N_CORES)
    Exp = mybir.ActivationFunctionType.Exp
    Copy = mybir.ActivationFunctionType.Copy

    xT8_d = nc.dram_tensor("xT8", [D, S], F8, kind="ExternalInput").ap()
    xTb_d = nc.dram_tensor("xTb", [D, QH], CDT, kind="ExternalInput").ap()
    xN_d = nc.dram_tensor("xN", [S, D], CDT, kind="ExternalInput").ap()
    m2_d = nc.dram_tensor("M2", [D, D], CDT, kind="ExternalInput").ap()
    wvT_d = nc.dram_tensor("WvT", [D, D], CDT, kind="ExternalInput").ap()
    z_d = nc.dram_tensor("z2", [P, DT], F32, kind="ExternalInput").ap()
    bv_d = nc.dram_tensor("bvb", [P, D], F32, kind="ExternalInput").ap()
    out_d = nc.dram_tensor("out", [QH, D], F32, kind="ExternalOutput").ap()

    with tile.TileContext(nc) as tc:
        with (
            tc.tile_pool(name="resident", bufs=1) as res,
            tc.tile_pool(name="pt", bufs=2) as ptpool,
            tc.tile_pool(name="tm", bufs=2) as tmpool,
            tc.tile_pool(name="osb", bufs=4) as opool,
            tc.tile_pool(name="small", bufs=4) as spool,
            tc.tile_pool(name="ps", bufs=4, space="PSUM") as psA,
            tc.tile_pool(name="ptm", bufs=3, space="PSUM") as psT,
            tc.tile_pool(name="pss", bufs=1, space="PSUM") as psS,
        ):
            # ---- resident loads (once) ----
            # Order matters for the single-shot prologue: the Y matmuls
            # need z + M2 + xTb first; xt2 (pass1 lhsT) next; xN (pass2)
            # and bv (epilogue) last. Loads alternate between the sync
            # and scalar HWDGE queues.
            z_sb = res.tile([P, DT], F32, tag="z", name="z_sb")
            nc.scalar.dma_start(z_sb[:], z_d[:, :])
            m2 = [res.tile([P, D], CDT, tag=f"m{d}", name=f"m2_{d}")
                  for d in range(DT)]
            for d in range(DT):
                (nc.sync if d % 2 else nc.scalar).dma_start(
                    m2[d][:], m2_d[d * P:(d + 1) * P, :])
            xtb = [res.tile([P, QH], CDT, tag=f"xb{d}", name=f"xb{d}")
                   for d in range(DT)]
            for half in range(2):
                cols = slice(half * 512, (half + 1) * 512)
                for d in range(DT):
                    (nc.sync if d % 2 else nc.scalar).dma_start(
                        xtb[d][:, cols], xTb_d[d * P:(d + 1) * P, cols])
            xt2 = [res.tile([P, 2, S], F8, tag=f"xt{dp}", name=f"xt{dp}")
                   for dp in range(DP)]
            # pass1 reads xt2 k-tiles in order: land the first column
            # half of every (dp, j) row before any second half, so the
            # first q-block's score matmuls can start sooner.
            for half in range(2):
                cols = slice(half * QH, (half + 1) * QH)
                for dp in range(DP):
                    for j in range(2):
                        nc.gpsimd.dma_start(
                            xt2[dp][:, j, cols],
                            xT8_d[(2 * dp + j) * P:(2 * dp + j + 1) * P,
                                  cols])
            xn = [res.tile([P, D], CDT, tag=f"xn{k}", name=f"xn{k}")
                  for k in range(KT_N)]
            bv_sb = res.tile([P, D], F32, tag="bv", name="bv_sb")
            _xnq = [nc.sync, nc.scalar, nc.gpsimd]
            for k in range(KT_N):
                _xnq[k % 3].dma_start(xn[k][:], xN_d[k * P:(k + 1) * P, :])
            nc.scalar.dma_start(bv_sb[:], bv_d[:, :])
            wv = [res.tile([P, D], CDT, tag=f"w{d}", name=f"wv_{d}")
                  for d in range(DT)]
            for d in range(DT):
                (nc.sync if d % 2 else nc.scalar).dma_start(
                    wv[d][:], wvT_d[d * P:(d + 1) * P, :])
            ones = res.tile([P, 1], CDT, tag="ones", name="ones")
            nc.vector.memset(ones[:], 1.0)
            onesF = res.tile([P, 1], F32, tag="onesF", name="onesF")
            nc.vector.memset(onesF[:], 1.0)
            one11 = res.tile([1, 1], F32, tag="one11", name="one11")
            nc.vector.memset(one11[:], 1.0)

            yt2 = [res.tile([P, 2, QH], F8, tag=f"yt{dp}", name=f"yt{dp}")
                   for dp in range(DP)]

            # PE warm-up: ~4.5us of dependency-free matmuls (memset
            # operands, no DMA deps) keep the PE busy from t=0 so the
            # HAM clock gate is at 8/8 when the first Y matmul issues;
            # the PE would otherwise idle here waiting on the prologue
            # DMAs anyway.
            warm = res.tile([P, P], CDT, tag="warm", name="warm")
            nc.vector.memset(warm[:], 0.5)
            wps = psS.tile([P, QB], F32, tag="pss", name="wps")
            for _w in range(40):
                nc.tensor.matmul(wps[0:1, 0:P], lhsT=warm[:, 0:1],
                                 rhs=warm[:, :], start=True, stop=True,
                                 skip_group_check=True)

            def y_phase():
                for sb in range(QH // 512):
                    for dc in range(DT):
                        ps = psA.tile([P, 512], F32, tag="ps", name="ps")
                        for d in range(DT):
                            nc.tensor.matmul(
                                ps[:],
                                lhsT=m2[d][:, dc * P:(dc + 1) * P],
                                rhs=xtb[d][:, sb * 512:(sb + 1) * 512],
                                start=(d == 0), stop=(d == DT - 1))
                        # yt = 128*Y + z2 in fp8 (z2 host-scaled by 128).
                        # On DVE so ScalarE stays Exp-only (no act-table
                        # reloads between functions).
                        nc.vector.tensor_scalar(
                            out=yt2[dc // 2][:, dc % 2,
                                             sb * 512:(sb + 1) * 512],
                            in0=ps[:], scalar1=Y_SCALE,
                            scalar2=z_sb[:, dc:dc + 1],
                            op0=mybir.AluOpType.mult,
                            op1=mybir.AluOpType.add)

            def pass1(qb):
                # ---- pass 1: scores (fp8 DR), exp, DVE row sums ----
                if packed_rowsum:
                    srow_ps = psS.tile([P, QB], F32, tag="pss",
                                       name="srow_ps")
                pts = []
                for k in range(KT_N):
                    psa = psA.tile([P, QB], F32, tag="ps", name="psa")
                    for dp in range(DP):
                        nc.tensor.matmul(
                            psa[:],
                            lhsT=xt2[dp][:, :, k * P:(k + 1) * P],
                            rhs=yt2[dp][:, :, qb * QB:(qb + 1) * QB],
                            start=(dp == 0), stop=(dp == DP - 1),
                            perf_mode=DR)
                    pt_sb = ptpool.tile([P, QB], CDT, tag=f"pt{k}",
                                        name=f"pt_sb{k}")
                    # PT = exp(psa/128)
                    nc.scalar.activation(pt_sb[:], psa[:], Exp,
                                         scale=1.0 / Y_SCALE)
                    pts.append(pt_sb)
                accb = None
                if packed_rowsum:
                    # rowsum: 4 col-group-packed chains of 4 k-tiles
                    # each; group g accumulates into partition 32*g.
                    for g in range(4):
                        for i in range(4):
                            k = 4 * g + i
                            nc.tensor.matmul(
                                srow_ps[32 * g:32 * g + 1, :],
                                lhsT=ones[:], rhs=pts[k][:],
                                start=(i == 0), stop=(i == 3),
                                tile_position=(0, 32 * g),
                                skip_group_check=True)
                    srow_sb = spool.tile([P, QB], F32, tag="srow",
                                         name="srow_sb")
                    for g in range(4):
                        nc.vector.tensor_copy(
                            out=srow_sb[32 * g:32 * g + 1, :],
                            in_=srow_ps[32 * g:32 * g + 1, :])
                    recs = []
                    for qs in range(QS):
                        scol_ps = psA.tile([P, 1], F32, tag="ps",
                                           name="scol_ps")
                        # sum the 4 group partials while transposing:
                        # 4 accumulating K=1 matmuls at row groups 32g
                        for g in range(4):
                            nc.tensor.matmul(
                                scol_ps[:],
                                lhsT=srow_sb[32 * g:32 * g + 1,
                                             qs * P:(qs + 1) * P],
                                rhs=onesF[32 * g:32 * g + 1, :],
                                start=(g == 0), stop=(g == 3),
                                tile_position=(32 * g, 0),
                                skip_group_check=True)
                        rec = spool.tile([P, 1], F32, tag="rec",
                                         name="rec")
                        nc.vector.reciprocal(rec[:], scol_ps[:])
                        recs.append(rec)
                else:
                    # Accumulate the 16 PT tiles elementwise on DVE
                    # (f32; final add emits a bf16 copy so the
                    # partition-reduce matmul runs at bf16 rate).
                    # Keeps 15 N=512 matmuls off the PE. The matmul
                    # part of the reduction is emitted later, AFTER
                    # pass2 (rowsum_reduce), so the PE never waits on
                    # this DVE chain.
                    acc = spool.tile([P, QB], F32, tag="acc", name="acc")
                    nc.vector.tensor_add(acc[:], pts[0][:], pts[1][:])
                    for k in range(2, KT_N - 1):
                        nc.vector.tensor_add(acc[:], acc[:], pts[k][:])
                    accb = spool.tile([P, QB], CDT, tag="accb",
                                      name="accb")
                    nc.vector.tensor_add(accb[:], acc[:],
                                         pts[KT_N - 1][:])
                    recs = None
                return pts, accb, recs

            def rowsum_reduce(accb):
                # partition-reduce the DVE-accumulated exp sums (bf16
                # ones-matmul), transpose per 128-q block, reciprocal.
                srow_ps = psS.tile([P, QB], F32, tag="pss",
                                   name="srow_ps")
                nc.tensor.matmul(
                    srow_ps[0:1, :], lhsT=ones[:], rhs=accb[:],
                    start=True, stop=True)
                srow_sb = spool.tile([1, QB], F32, tag="srow",
                                     name="srow_sb")
                nc.vector.tensor_copy(out=srow_sb[:],
                                      in_=srow_ps[0:1, :])
                recs = []
                for qs in range(QS):
                    scol_ps = psA.tile([P, 1], F32, tag="ps",
                                       name="scol_ps")
                    nc.tensor.matmul(
                        scol_ps[:],
                        lhsT=srow_sb[0:1, qs * P:(qs + 1) * P],
                        rhs=one11[:], start=True, stop=True)
                    rec = spool.tile([P, 1], F32, tag="rec",
                                     name="rec")
                    nc.vector.reciprocal(rec[:], scol_ps[:])
                    recs.append(rec)
                return recs

            def pass2(pts):
                # ---- pass 2: tmpT[d, q] = sum_k x_k^T P^T (bf16) ----
                tms = []
                for dt_i in range(DT):
                    pst = psT.tile([P, 512], F32, tag="ptm", name="pst")
                    for k in range(KT_N):
                        nc.tensor.matmul(
                            pst[:],
                            lhsT=xn[k][:, dt_i * P:(dt_i + 1) * P],
                            rhs=pts[k][:],
                            start=(k == 0), stop=(k == KT_N - 1))
                    tm = tmpool.tile([P, 512], CDT, tag=f"tm{dt_i}",
                                     name=f"tm{dt_i}")
                    nc.vector.tensor_copy(out=tm[:], in_=pst[:])
                    tms.append(tm)
                return tms

            def pass3(qb, tms, recs):
                # ---- pass 3: out[q, e] = tmpT^T @ WvT, scaled + bv ----
                for qs in range(QS):
                    for eb in range(2):
                        pso = psA.tile([P, 512], F32, tag="ps",
                                       name="pso")
                        for dt_i in range(DT):
                            nc.tensor.matmul(
                                pso[:],
                                lhsT=tms[dt_i][:, qs * P:(qs + 1) * P],
                                rhs=wv[dt_i][:, eb * 512:(eb + 1) * 512],
                                start=(dt_i == 0), stop=(dt_i == DT - 1))
                        osb = opool.tile([P, 512], F32, tag="osb",
                                         name="osb")
                        # out = pso * (1/S) + bv, fused on DVE
                        nc.vector.scalar_tensor_tensor(
                            out=osb[:], in0=pso[:], scalar=recs[qs][:],
                            in1=bv_sb[:, eb * 512:(eb + 1) * 512],
                            op0=mybir.AluOpType.mult,
                            op1=mybir.AluOpType.add)
                        row = qb * QB + qs * P
                        (nc.sync if eb else nc.gpsimd).dma_start(
                            out_d[row:row + P, eb * 512:(eb + 1) * 512],
                            osb[:])

            a_reps = reps if mode in ("full", "A") else 1
            b_reps = reps if mode in ("full", "B") else 1
            for _i in range(a_reps):
                y_phase()
            for _i in range(b_reps):
                for qb in range(NQB):
                    pts, accb, recs = pass1(qb)
                    if mode == "P1" and qb == 0:
                        for _j in range(reps - 1):
                            pts, accb, recs = pass1(0)
                    tms = pass2(pts)
                    if mode == "P2" and qb == 0:
                        for _j in range(reps - 1):
                            tms = pass2(pts)
                    if recs is None:
                        recs = rowsum_reduce(accb)
                    pass3(qb, tms, recs)
                    if mode == "P3" and qb == 0:
                        for _j in range(reps - 1):
                            pass3(0, tms, recs)
            if mode == "A":
                nc.gpsimd.dma_start(out_d[0:P, 0:8], yt2[0][:, 0, 0:8])
    nc.compile()
    return nc


def _get_nc(reps: int = 1, mode: str = "full"):
    key = (reps, mode)
    if key not in _NC_CACHE:
        _NC_CACHE[key] = build_nc(reps, mode)
    return _NC_CACHE[key]


def make_in_maps(x, Wq, bq, Wk, bk, Wv, bv):
    inv = np.float64(1.0 / np.sqrt(D))
    M2 = Wq.T.astype(np.float64) @ Wk.astype(np.float64) * inv
    z = Wk.T.astype(np.float64) @ bq.astype(np.float64) * inv
    m2b = np.ascontiguousarray(M2.astype(np.float32)).astype(BF16)
    wvT = np.ascontiguousarray(Wv.T).astype(BF16)
    z2 = np.ascontiguousarray(
        (z * Y_SCALE).astype(np.float32).reshape(DT, P).T).astype(np.float32)
    bvb = np.ascontiguousarray(np.broadcast_to(bv, (P, D))).astype(np.float32)
    in_maps = []
    for c in range(N_CORES):
        b, h = divmod(c, 2)
        # rotate the sequence axis so this core's query half is always
        # columns/rows 0:QH -- attention is permutation-invariant along
        # the key axis as long as xT (pass1) and xN (pass2) agree.
        xr = np.roll(x[b], -h * QH, axis=0)
        xT = np.ascontiguousarray(xr.T)
        xT8 = xT.astype(FP8)
        xTb = np.ascontiguousarray(xT[:, :QH]).astype(BF16)
        xN = np.ascontiguousarray(xr).astype(BF16)
        in_maps.append({
            "xT8": xT8, "xTb": xTb, "xN": xN,
            "M2": m2b, "WvT": wvT,
            "z2": z2, "bvb": bvb,
        })
    return in_maps


def kernel(x, Wq, bq, Wk, bk, Wv, bv):
    x = np.asarray(x, np.float32)
    in_maps = make_in_maps(x, np.asarray(Wq, np.float32),
                           np.asarray(bq, np.float32),
                           np.asarray(Wk, np.float32),
                           np.asarray(bk, np.float32),
                           np.asarray(Wv, np.float32),
                           np.asarray(bv, np.float32))
    nc = _get_nc()
    res = run_bass_kernel_spmd(nc, in_maps, core_ids=list(range(N_CORES)))
    out = np.empty((B, S, D), np.float32)
    for c in range(N_CORES):
        b, h = divmod(c, 2)
        out[b, h * QH:(h + 1) * QH, :] = res.results[c]["out"]
    return out


# revision 36
# speedup vs baseline: 5.8656x; 5.1980x over previous
"""Fused single-head attention kernel for 8 TRN2 NeuronCores.

Problem: B=4, S=2048, D=1024 attention:
    Q = x @ Wq.T + bq; K = x @ Wk.T + bk; V = x @ Wv.T + bv
    out = softmax(Q K^T / sqrt(D)) @ V

Sharding (no cross-core traffic): core c handles batch b = c//2 and
query half h = c%2 (1024 queries).

Algebraic refactoring: the K/V projections are folded away:
  logits[k,q] = x_k . (M2^T x_q + z)   (M2 = Wq^T Wk / sqrt(D),
                                        z = Wk^T bq / sqrt(D);
  per-query terms drop out of the softmax). out = ((P x) Wv^T)/rowsum(P)
  + bv, so V is never materialized.

The S x QH logits matmul (pass1, the largest single score-path item)
runs in fp8-e4m3 with MatmulPerfMode.DoubleRow (K=256 per matmul,
2 fp8 weights per PE cell), roughly halving its TensorE time. Y itself
is computed in bf16 (fp8 M2/x there pushes rel-err past the 2e-2 gate:
measured 2.1e-2 full-fp8 vs 1.4e-2 with bf16 Y; bf16 baseline 2.3e-3).
The output path (pass2: x^T P, pass3: (.)Wv^T) stays bf16.

fp8 scale management (e4m3 min normal 2^-6): Y has rms ~0.01, so the
Y evacuation scales by 128 (-> rms ~1.3 in fp8; bias z is host-scaled
by 128 and added in the same activation, before quantization). pass1
PSUM then holds 128*logits and the exp evac uses scale 1/128.

DoubleRow data layout: 3D tiles [128, 2, N]; element (p, j, n) is
contraction index (2*dp+j)*128+p for d-tile-pair dp. Both operands use
the same (p, j) pairing; out = sum_j lhsT[:,j,:].T @ rhs[:,j,:].

Engine balance: the PE is the bottleneck (~650 matmul instructions at
the HW-measured ~255-265 ns for a chained self-loading N=512 matmul),
so everything else is kept off it: the softmax rowsum is an
elementwise DVE accumulation of the 16 exp tiles plus ONE fp32
ones-matmul for the partition reduction; all PSUM evacuations are DVE
(fused scale+bias via tensor_scalar / scalar_tensor_tensor), leaving
ScalarE to run Exp only (also avoids act-table reloads). DMA rides all
three queues (sync/SP, scalar/ACT, gpsimd). A ~4.5us burst of
dependency-free warm-up matmuls at t=0 brings the PE HAM clock gate to
8/8 while the prologue DMAs stream.

(The 4-way col-group tile_position packing of the rowsum - kept behind
packed_rowsum=True - is numerically correct in CoreSim and in an
isolated HW micro-test but corrupts results at ~1.8e-1 rel err when
its packed matmuls interleave with full-array matmuls in the
production kernel, so it stays off.)

Device dataflow:
  Y[dc,q] = M2^T.T @ xqT (bf16, 128 MMs); DVE evac scale 128 + 128z
    -> yt2 fp8 [128,2,QH] x4
  per q-block of 512 (pass1/2/3):
    pass1: attT[k,q] += xt2^T.T @ yt2 (4 fp8-DR MMs per k-tile);
      PT = exp(psa/128) (ScalarE, PSUM->SBUF bf16);
      rowsum: DVE accumulates the 16 PT tiles (f32), one onesF fp32
      matmul reduces partitions -> [1,512]; per 128-q block a tiny
      [1,128]^T@[1,1] matmul transposes, feeding the DVE reciprocal
    pass2: tmpT[d,q] += xN_slice^T.T @ PT (bf16, PSUM over 16 k-tiles)
    pass3: out[q,e] += tmpT_slice^T.T @ WvT (bf16); DVE evac fuses
      out = out * (1/S) + bv -> DMA to DRAM

All weights (M2, WvT) and x layouts are SBUF-resident after the
prologue; the steady-state loop does no input DMA.

Measured on the axon TRN2 pod: ~164.8 us/rep (slope method) vs
219.1 us for the all-bf16 baseline; per-phase marginals (Y 32.9,
pass1 16.1, pass2 30.5, pass3 16.9 us) each sit at the HW
instruction floor of ~230-265 ns per N=512 matmul. rel err
1.402e-2 (gate 2e-2), matching the numpy fp8 error model.
"""

import os
import sys

for _p in ("/opt/trn_rl_repo", "/root/.axon_site/_ro/trn_rl_repo"):
    if os.path.isdir(_p) and _p not in sys.path:
        sys.path.insert(0, _p)

import numpy as np
import ml_dtypes

import concourse.bass as bass
import concourse.tile as tile
from concourse import bacc, mybir
from concourse.bass_utils import run_bass_kernel_spmd

BF16 = ml_dtypes.bfloat16
FP8 = ml_dtypes.float8_e4m3
F32 = mybir.dt.float32
CDT = mybir.dt.bfloat16
F8 = mybir.dt.float8e4
DR = mybir.MatmulPerfMode.DoubleRow

B, S, D = 4, 2048, 1024
N_CORES = 8
P = 128
DT = D // P          # 8 d-tiles (contraction)
DP = DT // 2         # 4 d-tile PAIRS for DoubleRow
KT_N = S // P        # 16 k-tiles
QH = S // 2          # 1024 queries per core
QB = 512             # q-block for phase B
NQB = QH // QB       # 2 q-blocks
QS = QB // P         # 4 q-subtiles per block

Y_SCALE = 128.0      # scale of Y as stored in fp8

_NC_CACHE = {}


def build_nc(reps: int = 1, mode: str = "full", *, dr: bool = True,
             packed_rowsum: bool = False):
    # packed_rowsum (col-group tile_position packing) measures 1.8e-1
    # rel err on real HW despite being correct in CoreSim and in an
    # isolated micro-test -- some PE weight-buffer hazard when packed
    # matmuls interleave with full-array matmuls. Keep it off.
    nc = bacc.Bacc("TRN2", target_bir_lowering=False, debug=False,
                   num_devices=N_CORES)
    Exp = mybir.ActivationFunctionType.Exp
    Copy = mybir.ActivationFunctionType.Copy

    xT8_d = nc.dram_tensor("xT8", [D, S], F8, kind="ExternalInput").ap()
    xTb_d = nc.dram_tensor("xTb", [D, QH], CDT, kind="ExternalInput").ap()
    xN_d = nc.dram_tensor("xN", [S, D], CDT, kind="ExternalInput").ap()
    m2_d = nc.dram_tensor("M2", [D, D], CDT, kind="ExternalInput").ap()
    wvT_d = nc.dram_tensor("WvT", [D, D], CDT, kind="ExternalInput").ap()
    z_d = nc.dram_tensor("z2", [P, DT], F32, kind="ExternalInput").ap()
    bv_d = nc.dram_tensor("bvb", [P, D], F32, kind="ExternalInput").ap()
    out_d = nc.dram_tensor("out", [QH, D], F32, kind="ExternalOutput").ap()

    with tile.TileContext(nc) as tc:
        with (
            tc.tile_pool(name="resident", bufs=1) as res,
            tc.tile_pool(name="pt", bufs=2) as ptpool,
            tc.tile_pool(name="tm", bufs=2) as tmpool,
            tc.tile_pool(name="osb", bufs=4) as opool,
            tc.tile_pool(name="small", bufs=4) as spool,
            tc.tile_pool(name="ps", bufs=4, space="PSUM") as psA,
            tc.tile_pool(name="ptm", bufs=3, space="PSUM") as psT,
            tc.tile_pool(name="pss", bufs=1, space="PSUM") as psS,
        ):
            # ---- resident loads (once) ----
            # Order matters for the single-shot prologue: the Y matmuls
            # need z + M2 + xTb first; xt2 (pass1 lhsT) next; xN (pass2)
            # and bv (epilogue) last. Loads alternate between the sync
            # and scalar HWDGE queues.
            z_sb = res.tile([P, DT], F32, tag="z", name="z_sb")
            nc.scalar.dma_start(z_sb[:], z_d[:, :])
            m2 = [res.tile([P, D], CDT, tag=f"m{d}", name=f"m2_{d}")
                  for d in range(DT)]
            for d in range(DT):
                (nc.sync if d % 2 else nc.scalar).dma_start(
                    m2[d][:], m2_d[d * P:(d + 1) * P, :])
            xtb = [res.tile([P, QH], CDT, tag=f"xb{d}", name=f"xb{d}")
                   for d in range(DT)]
            for half in range(2):
                cols = slice(half * 512, (half + 1) * 512)
                for d in range(DT):
                    (nc.sync if d % 2 else nc.scalar).dma_start(
                        xtb[d][:, cols], xTb_d[d * P:(d + 1) * P, cols])
            xt2 = [res.tile([P, 2, S], F8, tag=f"xt{dp}", name=f"xt{dp}")
                   for dp in range(DP)]
            # pass1 reads xt2 k-tiles in order: land the first column
            # half of every (dp, j) row before any second half, so the
            # first q-block's score matmuls can start sooner.
            for half in range(2):
                cols = slice(half * QH, (half + 1) * QH)
                for dp in range(DP):
                    for j in range(2):
                        nc.gpsimd.dma_start(
                            xt2[dp][:, j, cols],
                            xT8_d[(2 * dp + j) * P:(2 * dp + j + 1) * P,
                                  cols])
            xn = [res.tile([P, D], CDT, tag=f"xn{k}", name=f"xn{k}")
                  for k in range(KT_N)]
            bv_sb = res.tile([P, D], F32, tag="bv", name="bv_sb")
            _xnq = [nc.sync, nc.scalar, nc.gpsimd]
            for k in range(KT_N):
                _xnq[k % 3].dma_start(xn[k][:], xN_d[k * P:(k + 1) * P, :])
            nc.scalar.dma_start(bv_sb[:], bv_d[:, :])
            wv = [res.tile([P, D], CDT, tag=f"w{d}", name=f"wv_{d}")
                  for d in range(DT)]
            for d in range(DT):
                (nc.sync if d % 2 else nc.scalar).dma_start(
                    wv[d][:], wvT_d[d * P:(d + 1) * P, :])
            ones = res.tile([P, 1], CDT, tag="ones", name="ones")
            nc.vector.memset(ones[:], 1.0)
            onesF = res.tile([P, 1], F32, tag="onesF", name="onesF")
            nc.vector.memset(onesF[:], 1.0)
            one11 = res.tile([1, 1], F32, tag="one11", name="one11")
            nc.vector.memset(one11[:], 1.0)

            yt2 = [res.tile([P, 2, QH], F8, tag=f"yt{dp}", name=f"yt{dp}")
                   for dp in range(DP)]

            # PE warm-up: ~4.5us of dependency-free matmuls (memset
            # operands, no DMA deps) keep the PE busy from t=0 so the
            # HAM clock gate is at 8/8 when the first Y matmul issues;
            # the PE would otherwise idle here waiting on the prologue
            # DMAs anyway.
            warm = res.tile([P, P], CDT, tag="warm", name="warm")
            nc.vector.memset(warm[:], 0.5)
            wps = psS.tile([P, QB], F32, tag="pss", name="wps")
            for _w in range(40):
                nc.tensor.matmul(wps[0:1, 0:P], lhsT=warm[:, 0:1],
                                 rhs=warm[:, :], start=True, stop=True,
                                 skip_group_check=True)

            def y_phase():
                for sb in range(QH // 512):
                    for dc in range(DT):
                        ps = psA.tile([P, 512], F32, tag="ps", name="ps")
                        for d in range(DT):
                            nc.tensor.matmul(
                                ps[:],
                                lhsT=m2[d][:, dc * P:(dc + 1) * P],
                                rhs=xtb[d][:, sb * 512:(sb + 1) * 512],
                                start=(d == 0), stop=(d == DT - 1))
                        # yt = 128*Y + z2 in fp8 (z2 host-scaled by 128).
                        # On DVE so ScalarE stays Exp-only (no act-table
                        # reloads between functions).
                        nc.vector.tensor_scalar(
                            out=yt2[dc // 2][:, dc % 2,
                                             sb * 512:(sb + 1) * 512],
                            in0=ps[:], scalar1=Y_SCALE,
                            scalar2=z_sb[:, dc:dc + 1],
                            op0=mybir.AluOpType.mult,
                            op1=mybir.AluOpType.add)

            def pass1(qb):
                # ---- pass 1: scores (fp8 DR), exp, DVE row sums ----
                if packed_rowsum:
                    srow_ps = psS.tile([P, QB], F32, tag="pss",
                                       name="srow_ps")
                pts = []
                for k in range(KT_N):
                    psa = psA.tile([P, QB], F32, tag="ps", name="psa")
                    for dp in range(DP):
                        nc.tensor.matmul(
                            psa[:],
                            lhsT=xt2[dp][:, :, k * P:(k + 1) * P],
                            rhs=yt2[dp][:, :, qb * QB:(qb + 1) * QB],
                            start=(dp == 0), stop=(dp == DP - 1),
                            perf_mode=DR)
                    pt_sb = ptpool.tile([P, QB], CDT, tag=f"pt{k}",
                                        name=f"pt_sb{k}")
                    # PT = exp(psa/128)
                    nc.scalar.activation(pt_sb[:], psa[:], Exp,
                                         scale=1.0 / Y_SCALE)
                    pts.append(pt_sb)
                accb = None
                if packed_rowsum:
                    # rowsum: 4 col-group-packed chains of 4 k-tiles
                    # each; group g accumulates into partition 32*g.
                    for g in range(4):
                        for i in range(4):
                            k = 4 * g + i
                            nc.tensor.matmul(
                                srow_ps[32 * g:32 * g + 1, :],
                                lhsT=ones[:], rhs=pts[k][:],
                                start=(i == 0), stop=(i == 3),
                                tile_position=(0, 32 * g),
                                skip_group_check=True)
                    srow_sb = spool.tile([P, QB], F32, tag="srow",
                                         name="srow_sb")
                    for g in range(4):
                        nc.vector.tensor_copy(
                            out=srow_sb[32 * g:32 * g + 1, :],
                            in_=srow_ps[32 * g:32 * g + 1, :])
                    recs = []
                    for qs in range(QS):
                        scol_ps = psA.tile([P, 1], F32, tag="ps",
                                           name="scol_ps")
                        # sum the 4 group partials while transposing:
                        # 4 accumulating K=1 matmuls at row groups 32g
                        for g in range(4):
                            nc.tensor.matmul(
                                scol_ps[:],
                                lhsT=srow_sb[32 * g:32 * g + 1,
                                             qs * P:(qs + 1) * P],
                                rhs=onesF[32 * g:32 * g + 1, :],
                                start=(g == 0), stop=(g == 3),
                                tile_position=(32 * g, 0),
                                skip_group_check=True)
                        rec = spool.tile([P, 1], F32, tag="rec",
                                         name="rec")
                        nc.vector.reciprocal(rec[:], scol_ps[:])
                        recs.append(rec)
                else:
                    # Accumulate the 16 PT tiles elementwise on DVE
                    # (f32; final add emits a bf16 copy so the
                    # partition-reduce matmul runs at bf16 rate).
                    # Keeps 15 N=512 matmuls off the PE. The matmul
                    # part of the reduction is emitted later, AFTER
                    # pass2 (rowsum_reduce), so the PE never waits on
                    # this DVE chain.
                    acc = spool.tile([P, QB], F32, tag="acc", name="acc")
                    nc.vector.tensor_add(acc[:], pts[0][:], pts[1][:])
                    for k in range(2, KT_N - 1):
                        nc.vector.tensor_add(acc[:], acc[:], pts[k][:])
                    accb = spool.tile([P, QB], CDT, tag="accb",
                                      name="accb")
                    nc.vector.tensor_add(accb[:], acc[:],
                                         pts[KT_N - 1][:])
                    recs = None
                return pts, accb, recs

            def rowsum_reduce(accb):
                # partition-reduce the DVE-accumulated exp sums with
                # accb as the STATIONARY operand: accb_slice.T @ ones
                # yields the per-query sums already transposed to
                # [128,1] -- one tiny N=1 matmul per 128-q block.
                recs = []
                for qs in range(QS):
                    scol_ps = psA.tile([P, 1], F32, tag="ps",
                                       name="scol_ps")
                    nc.tensor.matmul(
                        scol_ps[:],
                        lhsT=accb[:, qs * P:(qs + 1) * P],
                        rhs=ones[:], start=True, stop=True)
                    rec = spool.tile([P, 1], F32, tag="rec",
                                     name="rec")
                    nc.vector.reciprocal(rec[:], scol_ps[:])
                    recs.append(rec)
                return recs

            def pass2(pts):
                # ---- pass 2: tmpT[d, q] = sum_k x_k^T P^T (bf16) ----
                tms = []
                for dt_i in range(DT):
                    pst = psT.tile([P, 512], F32, tag="ptm", name="pst")
                    for k in range(KT_N):
                        nc.tensor.matmul(
                            pst[:],
                            lhsT=xn[k][:, dt_i * P:(dt_i + 1) * P],
                            rhs=pts[k][:],
                            start=(k == 0), stop=(k == KT_N - 1))
                    tm = tmpool.tile([P, 512], CDT, tag=f"tm{dt_i}",
                                     name=f"tm{dt_i}")
                    nc.vector.tensor_copy(out=tm[:], in_=pst[:])
                    tms.append(tm)
                return tms

            def pass3(qb, tms, recs):
                # ---- pass 3: out[q, e] = tmpT^T @ WvT, scaled + bv ----
                for qs in range(QS):
                    for eb in range(2):
                        pso = psA.tile([P, 512], F32, tag="ps",
                                       name="pso")
                        for dt_i in range(DT):
                            nc.tensor.matmul(
                                pso[:],
                                lhsT=tms[dt_i][:, qs * P:(qs + 1) * P],
                                rhs=wv[dt_i][:, eb * 512:(eb + 1) * 512],
                                start=(dt_i == 0), stop=(dt_i == DT - 1))
                        osb = opool.tile([P, 512], F32, tag="osb",
                                         name="osb")
                        # out = pso * (1/S) + bv, fused on DVE
                        nc.vector.scalar_tensor_tensor(
                            out=osb[:], in0=pso[:], scalar=recs[qs][:],
                            in1=bv_sb[:, eb * 512:(eb + 1) * 512],
                            op0=mybir.AluOpType.mult,
                            op1=mybir.AluOpType.add)
                        row = qb * QB + qs * P
                        (nc.sync if eb else nc.gpsimd).dma_start(
                            out_d[row:row + P, eb * 512:(eb + 1) * 512],
                            osb[:])

            a_reps = reps if mode in ("full", "A") else 1
            b_reps = reps if mode in ("full", "B") else 1
            for _i in range(a_reps):
                y_phase()
            for _i in range(b_reps):
                for qb in range(NQB):
                    pts, accb, recs = pass1(qb)
                    if mode == "P1" and qb == 0:
                        for _j in range(reps - 1):
                            pts, accb, recs = pass1(0)
                    tms = pass2(pts)
                    if mode == "P2" and qb == 0:
                        for _j in range(reps - 1):
                            tms = pass2(pts)
                    if recs is None:
                        recs = rowsum_reduce(accb)
                    pass3(qb, tms, recs)
                    if mode == "P3" and qb == 0:
                        for _j in range(reps - 1):
                            pass3(0, tms, recs)
            if mode == "A":
                nc.gpsimd.dma_start(out_d[0:P, 0:8], yt2[0][:, 0, 0:8])
    nc.compile()
    return nc


def _get_nc(reps: int = 1, mode: str = "full"):
    key = (reps, mode)
    if key not in _NC_CACHE:
        _NC_CACHE[key] = build_nc(reps, mode)
    return _NC_CACHE[key]


def make_in_maps(x, Wq, bq, Wk, bk, Wv, bv):
    inv = np.float64(1.0 / np.sqrt(D))
    M2 = Wq.T.astype(np.float64) @ Wk.astype(np.float64) * inv
    z = Wk.T.astype(np.float64) @ bq.astype(np.float64) * inv
    m2b = np.ascontiguousarray(M2.astype(np.float32)).astype(BF16)
    wvT = np.ascontiguousarray(Wv.T).astype(BF16)
    z2 = np.ascontiguousarray(
        (z * Y_SCALE).astype(np.float32).reshape(DT, P).T).astype(np.float32)
    bvb = np.ascontiguousarray(np.broadcast_to(bv, (P, D))).astype(np.float32)
    in_maps = []
    for c in range(N_CORES):
        b, h = divmod(c, 2)
        # rotate the sequence axis so this core's query half is always
        # columns/rows 0:QH -- attention is permutation-invariant along
        # the key axis as long as xT (pass1) and xN (pass2) agree.
        xr = np.roll(x[b], -h * QH, axis=0)
        xT = np.ascontiguousarray(xr.T)
        xT8 = xT.astype(FP8)
        xTb = np.ascontiguousarray(xT[:, :QH]).astype(BF16)
        xN = np.ascontiguousarray(xr).astype(BF16)
        in_maps.append({
            "xT8": xT8, "xTb": xTb, "xN": xN,
            "M2": m2b, "WvT": wvT,
            "z2": z2, "bvb": bvb,
        })
    return in_maps


def kernel(x, Wq, bq, Wk, bk, Wv, bv):
    x = np.asarray(x, np.float32)
    in_maps = make_in_maps(x, np.asarray(Wq, np.float32),
                           np.asarray(bq, np.float32),
                           np.asarray(Wk, np.float32),
                           np.asarray(bk, np.float32),
                           np.asarray(Wv, np.float32),
                           np.asarray(bv, np.float32))
    nc = _get_nc()
    res = run_bass_kernel_spmd(nc, in_maps, core_ids=list(range(N_CORES)))
    out = np.empty((B, S, D), np.float32)
    for c in range(N_CORES):
        b, h = divmod(c, 2)
        out[b, h * QH:(h + 1) * QH, :] = res.results[c]["out"]
    return out
